# revision 11
# baseline (speedup 1.0000x reference)
"""Bilinear distance kernel for Trainium2 (8 NeuronCores, SPMD).

dists[b,n,m] = sum_{i,j} data[b,n,i] * W[0,i,j] * crit[b,m,j]
B=16, N=M=2048, LD=RD=128, fp32.

Sharding: data-parallel over B (2 batches per core). Per batch:
  dataT[i,n] , critT[j,m]  via PE transposes (contraction dim -> partitions)
  lwT[j,n]  = W.T @ dataT          (GEMM1, W stationary)
  out[n,m]  = lwT_tile.T @ critT   (GEMM2, fp32r full-rate)
Output writes (32 MiB/core) are the memory roofline; next-batch prep is
interleaved into GEMM2 emission so no engine sees a prep burst.
"""

import sys

if "/opt/trn_rl_repo" not in sys.path:
    sys.path.insert(0, "/opt/trn_rl_repo")

import numpy as np

B, N, M, D = 16, 2048, 2048, 128
NCORES = 8
BPC = B // NCORES  # batches per core

_cache = {}


def _build():
    if "nc" in _cache:
        return _cache["nc"]

    import concourse.bacc as bacc
    import concourse.mybir as mybir
    from concourse import tile

    f32 = mybir.dt.float32
    f32r = mybir.dt.float32r

    nc = bacc.Bacc()
    data_d = nc.dram_tensor("data", [BPC, N, D], f32, kind="ExternalInput")
    crit_d = nc.dram_tensor("crit", [BPC, M, D], f32, kind="ExternalInput")
    w_d = nc.dram_tensor("w", [D, D], f32, kind="ExternalInput")
    out_d = nc.dram_tensor("out", [BPC, N, M], f32, kind="ExternalOutput")
    ident_d = nc.inline_tensor(np.eye(D, dtype=np.float32), name="ident")

    NT = N // 128        # 16 n-tiles per batch
    LG = 8               # row-groups per load DMA (1 MiB loads)
    NL = N // (128 * LG)
    # store group sizes (n-tiles per store): small first groups to start
    # the store stream early; 2-tile (2 MiB) groups for the steady state.
    GROUPS = [1, 1] + [2] * 7

    store_rings = []  # filled after nc exists
    cp = {"k": 0, "tc": 0, "st": 0}

    with tile.TileContext(nc) as tc:
        store_rings = [nc.sync, nc.scalar, nc.gpsimd]
        with (
            tc.tile_pool(name="const", bufs=1) as cpool,
            tc.tile_pool(name="loads", bufs=3) as lpool,
            tc.tile_pool(name="big", bufs=2) as bigpool,
            tc.tile_pool(name="outs", bufs=3) as opool,
            tc.tile_pool(name="pst", bufs=3, space="PSUM") as pst,
            tc.tile_pool(name="psg", bufs=1, space="PSUM") as psg,
            tc.tile_pool(name="ps2", bufs=2, space="PSUM") as ps2,
        ):
            w_raw = cpool.tile([D, D], f32)
            nc.gpsimd.dma_start(w_raw[:], w_d[:])
            w_sb = cpool.tile([D, D], f32r)
            nc.vector.tensor_copy(w_sb[:], w_raw[:])
            ident = cpool.tile([D, D], f32)
            nc.gpsimd.dma_start(ident[:], ident_d[:])

            bigs = {}

            def alloc_big(b):
                bigs[b] = {
                    "dataT": bigpool.tile([D, N], f32r, tag="dataT", name=f"dataT{b}"),
                    "critT": bigpool.tile([D, M], f32r, tag="critT", name=f"critT{b}"),
                    "lwT": bigpool.tile([D, N], f32r, tag="lwT", name=f"lwT{b}"),
                }

            def tcopy_cast(dst_ap, src_ap):
                # alternate transpose-cast copies between DVE and ACT
                if cp["tc"] % 2 == 0:
                    nc.vector.tensor_copy(dst_ap, src_ap)
                else:
                    nc.scalar.copy(dst_ap, src_ap)
                cp["tc"] += 1

            def prep_thunks(b):
                """List of emission thunks for batch b's prep (loads,
                transposes, GEMM1), in dependency-friendly order."""
                th = []
                for src_d, key, ldeng in (
                    (crit_d, "critT", nc.sync),
                    (data_d, "dataT", nc.scalar),
                ):
                    for l in range(NL):
                        def load(b=b, src_d=src_d, key=key, l=l, ldeng=ldeng):
                            ld = lpool.tile([128, LG, D], f32, tag=key + "_ld")
                            bigs[b][key + f"_ld{l}"] = ld
                            ldeng.dma_start(
                                ld[:],
                                src_d[
                                    b, l * LG * 128 : (l + 1) * LG * 128, :
                                ].rearrange("(g p) d -> p g d", p=128),
                            )
                        th.append(load)
                        for g0 in range(0, LG, 4):
                            def tp(b=b, key=key, l=l, g0=g0):
                                ld = bigs[b][key + f"_ld{l}"]
                                dstT = bigs[b][key]
                                for g in range(g0, g0 + 4):
                                    ps = pst.tile([128, 128], f32, tag="pst")
                                    nc.tensor.transpose(ps[:], ld[:, g, :], ident[:])
                                    t = l * LG + g
                                    tcopy_cast(
                                        dstT[:, t * 128 : (t + 1) * 128], ps[:]
                                    )
                            th.append(tp)
                for c in range(N // 512):
                    def g1(b=b, c=c):
                        ps = psg.tile([128, 512], f32, tag="psg")
                        nc.tensor.matmul(
                            ps[:],
                            w_sb[:],
                            bigs[b]["dataT"][:, c * 512 : (c + 1) * 512],
                            start=True,
                            stop=True,
                        )
                        nc.vector.tensor_copy(
                            bigs[b]["lwT"][:, c * 512 : (c + 1) * 512], ps[:]
                        )
                    th.append(g1)
                return th

            def emit_gemm2(b, pending):
                """Emit batch b's GEMM2 + stores; after each store group,
                emit a couple of thunks from `pending` (next batch's prep)."""
                critT, lwT = bigs[b]["critT"], bigs[b]["lwT"]
                nt0 = 0
                per_group = (len(pending) + len(GROUPS) - 1) // len(GROUPS) if pending else 0
                for gi, sg in enumerate(GROUPS):
                    ot = opool.tile([128, 2, M], f32, tag="ot")
                    for ntl in range(sg):
                        nt = nt0 + ntl
                        lhs = lwT[:, nt * 128 : (nt + 1) * 128]
                        for h in range(2):
                            p2 = ps2.tile([128, 1024], f32, tag="ps2")
                            for q in range(2):
                                mc = h * 1024 + q * 512
                                nc.tensor.matmul(
                                    p2[:, q * 512 : (q + 1) * 512],
                                    lhs,
                                    critT[:, mc : mc + 512],
                                    start=True,
                                    stop=True,
                                )
                            if cp["k"] % 2 == 0:
                                nc.vector.tensor_copy(
                                    ot[:, ntl, h * 1024 : (h + 1) * 1024], p2[:]
                                )
                            else:
                                nc.scalar.copy(
                                    ot[:, ntl, h * 1024 : (h + 1) * 1024], p2[:]
                                )
                            cp["k"] += 1
                    ring = store_rings[cp["st"] % 3]
                    cp["st"] += 1
                    ring.dma_start(
                        out_d[
                            b, nt0 * 128 : (nt0 + sg) * 128, :
                        ].rearrange("(g p) m -> p g m", p=128),
                        ot[: , :sg, :],
                    )
                    nt0 += sg
                    # interleave next batch's prep
                    for _ in range(per_group):
                        if pending:
                            pending.pop(0)()
                while pending:
                    pending.pop(0)()

            alloc_big(0)
            for th in prep_thunks(0):
                th()
            for b in range(BPC):
                if b + 1 < BPC:
                    alloc_big(b + 1)
                    nxt = prep_thunks(b + 1)
                else:
                    nxt = []
                emit_gemm2(b, nxt)

    nc.finalize()
    _cache["nc"] = nc
    return nc


def kernel(data: np.ndarray, crit: np.ndarray, W: np.ndarray) -> np.ndarray:
    from concourse.bass_utils import run_bass_kernel_spmd

    nc = _build()
    data = np.ascontiguousarray(data, dtype=np.float32)
    crit = np.ascontiguousarray(crit, dtype=np.float32)
    w = np.ascontiguousarray(W.reshape(D, D), dtype=np.float32)
    in_maps = [
        {
            "data": data[c * BPC : (c + 1) * BPC],
            "crit": crit[c * BPC : (c + 1) * BPC],
            "w": w,
        }
        for c in range(NCORES)
    ]
    res = run_bass_kernel_spmd(nc, in_maps, core_ids=list(range(NCORES)))
    return np.concatenate([r["out"] for r in res.results], axis=0)


# revision 14
# speedup vs baseline: 1.0841x; 1.0841x over previous
"""Bilinear distance kernel for Trainium2 (8 NeuronCores, SPMD).

dists[b,n,m] = sum_{i,j} data[b,n,i] * W[0,i,j] * crit[b,m,j]
B=16, N=M=2048, LD=RD=128, fp32.

Sharding: data-parallel over B (2 batches per core). Per batch:
  dataT[i,n] , critT[j,m]  via PE transposes (contraction dim -> partitions)
  lwT[j,n]  = W.T @ dataT          (GEMM1, W stationary)
  out[n,m]  = lwT_tile.T @ critT   (GEMM2, fp32r full-rate)

Output writes (32 MiB/core) are the memory roofline. Engine roles keep the
store pipeline unblocked: DVE does only GEMM2 PSUM->SBUF copies (in store
order), ACT does prep casts + all load DMAs, stores rotate sync/gpsimd.
"""

import sys

if "/opt/trn_rl_repo" not in sys.path:
    sys.path.insert(0, "/opt/trn_rl_repo")

import numpy as np

B, N, M, D = 16, 2048, 2048, 128
NCORES = 8
BPC = B // NCORES  # batches per core

_cache = {}


def _build():
    if "nc" in _cache:
        return _cache["nc"]

    import concourse.bacc as bacc
    import concourse.mybir as mybir
    from concourse import tile

    f32 = mybir.dt.float32
    f32r = mybir.dt.float32r

    nc = bacc.Bacc()
    data_d = nc.dram_tensor("data", [BPC, N, D], f32, kind="ExternalInput")
    crit_d = nc.dram_tensor("crit", [BPC, M, D], f32, kind="ExternalInput")
    w_d = nc.dram_tensor("w", [D, D], f32, kind="ExternalInput")
    out_d = nc.dram_tensor("out", [BPC, N, M], f32, kind="ExternalOutput")
    ident_d = nc.inline_tensor(np.eye(D, dtype=np.float32), name="ident")

    LG = 8               # row-groups per load DMA (1 MiB loads)
    NL = N // (128 * LG)
    # store group sizes (n-tiles per store DMA): small groups at the ends
    # (fast fill / short drain), 2-tile (2 MiB) groups in the steady state.
    GROUPS = [1, 1, 2, 2, 2, 2, 2, 2, 1, 1]
    assert sum(GROUPS) == N // 128

    cp = {"st": 0}

    with tile.TileContext(nc) as tc:
        store_rings = [nc.sync, nc.gpsimd]
        with (
            tc.tile_pool(name="const", bufs=1) as cpool,
            tc.tile_pool(name="loads", bufs=4) as lpool,
            tc.tile_pool(name="big", bufs=2) as bigpool,
            tc.tile_pool(name="outs", bufs=3) as opool,
            tc.tile_pool(name="pst", bufs=3, space="PSUM") as pst,
            tc.tile_pool(name="psg", bufs=1, space="PSUM") as psg,
            tc.tile_pool(name="ps2", bufs=2, space="PSUM") as ps2,
        ):
            w_raw = cpool.tile([D, D], f32)
            nc.gpsimd.dma_start(w_raw[:], w_d[:])
            w_sb = cpool.tile([D, D], f32r)
            nc.scalar.copy(w_sb[:], w_raw[:])
            ident = cpool.tile([D, D], f32)
            nc.gpsimd.dma_start(ident[:], ident_d[:])

            bigs = {}

            def alloc_big(b):
                bigs[b] = {
                    "dataT": bigpool.tile([D, N], f32r, tag="dataT", name=f"dataT{b}"),
                    "critT": bigpool.tile([D, M], f32r, tag="critT", name=f"critT{b}"),
                    "lwT": bigpool.tile([D, N], f32r, tag="lwT", name=f"lwT{b}"),
                }

            lds = {}

            def load(b):
                """Issue batch b's load DMAs (crit then data) on the ACT ring."""
                alloc_big(b)
                for src_d, key in ((crit_d, "critT"), (data_d, "dataT")):
                    for l in range(NL):
                        ld = lpool.tile(
                            [128, LG, D], f32, tag=key + "_ld", name=f"{key}_ld{b}{l}"
                        )
                        lds[(b, key, l)] = ld
                        nc.scalar.dma_start(
                            ld[:],
                            src_d[
                                b, l * LG * 128 : (l + 1) * LG * 128, :
                            ].rearrange("(g p) d -> p g d", p=128),
                        )

            def prep(b):
                """Transposes and GEMM1 for batch b (loads already issued).
                crit first (GEMM2 needs all of critT). Casts on ACT."""
                for src_d, key in ((crit_d, "critT"), (data_d, "dataT")):
                    dstT = bigs[b][key]
                    for l in range(NL):
                        ld = lds[(b, key, l)]
                        for g in range(LG):
                            ps = pst.tile([128, 128], f32, tag="pst", name="pst")
                            nc.tensor.transpose(ps[:], ld[:, g, :], ident[:])
                            t = l * LG + g
                            nc.scalar.copy(dstT[:, t * 128 : (t + 1) * 128], ps[:])
                    if key == "dataT":
                        for c in range(N // 512):
                            ps = psg.tile([128, 512], f32, tag="psg", name="psg")
                            nc.tensor.matmul(
                                ps[:],
                                w_sb[:],
                                dstT[:, c * 512 : (c + 1) * 512],
                                start=True,
                                stop=True,
                            )
                            nc.scalar.copy(
                                bigs[b]["lwT"][:, c * 512 : (c + 1) * 512], ps[:]
                            )

            def gemm2_groups(b, lo, hi):
                """Emit GEMM2 store groups [lo, hi) for batch b."""
                critT, lwT = bigs[b]["critT"], bigs[b]["lwT"]
                nt0 = sum(GROUPS[:lo])
                for gi in range(lo, hi):
                    sg = GROUPS[gi]
                    ot = opool.tile([128, 2, M], f32, tag="ot", name="ot")
                    for ntl in range(sg):
                        nt = nt0 + ntl
                        lhs = lwT[:, nt * 128 : (nt + 1) * 128]
                        for h in range(2):
                            p2 = ps2.tile([128, 1024], f32, tag="ps2", name="ps2")
                            for q in range(2):
                                mc = h * 1024 + q * 512
                                nc.tensor.matmul(
                                    p2[:, q * 512 : (q + 1) * 512],
                                    lhs,
                                    critT[:, mc : mc + 512],
                                    start=True,
                                    stop=True,
                                )
                            nc.vector.tensor_copy(
                                ot[:, ntl, h * 1024 : (h + 1) * 1024], p2[:]
                            )
                    ring = store_rings[cp["st"] % 2]
                    cp["st"] += 1
                    ring.dma_start(
                        out_d[b, nt0 * 128 : (nt0 + sg) * 128, :].rearrange(
                            "(g p) m -> p g m", p=128
                        ),
                        ot[:, :sg, :],
                    )
                    nt0 += sg

            NG = len(GROUPS)
            for b in range(BPC):
                load(b)
            prep(0)
            gemm2_groups(0, 0, 4)
            for b in range(BPC):
                if b + 1 < BPC:
                    prep(b + 1)
                    gemm2_groups(b, 4, NG)
                    gemm2_groups(b + 1, 0, 4)
                else:
                    gemm2_groups(b, 4, NG)

    nc.finalize()
    _cache["nc"] = nc
    return nc


def kernel(data: np.ndarray, crit: np.ndarray, W: np.ndarray) -> np.ndarray:
    from concourse.bass_utils import run_bass_kernel_spmd

    nc = _build()
    data = np.ascontiguousarray(data, dtype=np.float32)
    crit = np.ascontiguousarray(crit, dtype=np.float32)
    w = np.ascontiguousarray(W.reshape(D, D), dtype=np.float32)
    in_maps = [
        {
            "data": data[c * BPC : (c + 1) * BPC],
            "crit": crit[c * BPC : (c + 1) * BPC],
            "w": w,
        }
        for c in range(NCORES)
    ]
    res = run_bass_kernel_spmd(nc, in_maps, core_ids=list(range(NCORES)))
    return np.concatenate([r["out"] for r in res.results], axis=0)
